# revision 23
# baseline (speedup 1.0000x reference)
"""FP8 delayed-scaling Linear (8192x4096 @ 4096x4096^T + bias) on 8 NeuronCores.

Strategy: 2D tensor-parallel sharding: token dim (T=8192) split 4 ways x
out_features (O=4096) split 2 ways -> 8 independent cores, no collectives
(the amax max-all-reduce happens in the host-side gather).

Numerics: the reference quantizes with OCP float8_e4m3fn (max 448). TRN2's
fp8e4 has max 240, so we quantize with the HALVED scale (s/2 = 224/amax) and
clip at +-224: every OCP grid point v with |v| <= 448 maps to v/2 which is
exactly representable in TRN fp8e4. The matmul output is then descaled by
4/(sx*sw) with the bias fused into PSUM eviction (scalar_tensor_tensor).

Schedule: one interleaved tape. The first 3 of 4 weight column-groups load
up front (right after the first token strip); each arriving strip then
unlocks three PSUM-bank jobs (~11 us of PE work vs ~10 us of DMA), so the
PE is the binding engine almost immediately. The 4th column group trickles
between strips, and its jobs are spliced in once it completes. Output
stores are spliced into the tape right at job completion so the sync-engine
DMA FIFO never holds finished evictions hostage behind pending input loads.
fp8 DoubleRow matmuls (256-deep contraction per instruction); LDWEIGHTS
hides in the PE reorder window.
"""

import numpy as np

import concourse.bass as bass
import concourse.bacc as bacc
import concourse.mybir as mybir
import concourse.tile as tile
from concourse import bass_utils

P = 128
FP8_MAX = 448.0
AMAX_EPS = 1e-8
MOMENTUM = 0.95
CLIP = 224.0  # 448/2 in scaled units

N_CORES = 8
A_SHARD = 4  # split of T (token rows)
B_SHARD = 2  # split of O (out features)


def _emission_tape(MT, NB, KP):
    """Emission tape: ("x", (m, half)) and ("w", (j, n)) input loads in
    arrival order, with whole ("job", (n, m)) entries spliced at the point
    both weight column-group n and strip m are resident.

    Weight group 0 loads solid right after the first two strips (first jobs
    start ~25us in); the remaining groups interleave 1:1 with strip pieces
    so each completion unlocks a growing backlog of jobs per strip."""
    events = []
    xq = [(m, h) for m in range(MT) for h in range(2)]
    events.extend(("x", xq.pop(0)) for _ in range(min(4, len(xq))))
    wq = [(j, n) for n in range(NB) for j in range(KP)]
    for _ in range(KP):  # group 0 solid
        events.append(("w", wq.pop(0)))
    while wq or xq:
        if xq:
            events.append(("x", xq.pop(0)))
        if wq:
            events.append(("w", wq.pop(0)))

    tape = []
    strips_done = []
    gcount = [0] * NB
    jobs = 0
    for kind, idx in events:
        tape.append((kind, idx))
        if kind == "x":
            m, h = idx
            if h == 1:
                strips_done.append(m)
                for n in range(NB):
                    if gcount[n] == KP:
                        tape.append(("job", (n, m)))
                        jobs += 1
        else:
            j, n = idx
            gcount[n] += 1
            if gcount[n] == KP:
                for m in strips_done:
                    tape.append(("job", (n, m)))
                    jobs += 1
    assert jobs == NB * MT, (jobs, NB, MT)
    return tape


def build_kernel(nc, K, T_loc, O_loc, sx2, sw2, descale):
    """Per-core kernel. Inputs (DRAM): xT [K, T_loc] f32, wT [K, O_loc] f32,
    bias [O_loc] f32. Outputs: out [T_loc, O_loc] f32,
    stats [128, 2] f32 (per-partition max|sx2*x|, max|sw2*w|)."""
    f32 = mybir.dt.float32
    f8 = mybir.dt.float8e4
    Alu = mybir.AluOpType
    Act = mybir.ActivationFunctionType

    assert K % (4 * P) == 0 and T_loc % P == 0 and O_loc % 512 == 0
    KS = K // P          # k-subtiles of 128
    KP = KS // 2         # k-pair slabs (DoubleRow consumes 2 subtiles)
    MT = T_loc // P      # t-strips
    NB = O_loc // 512    # 512-wide n-blocks (one PSUM bank each)
    KH = KS // 2         # k-subtiles per x staging piece

    xT = nc.dram_tensor("xT", (K, T_loc), f32, kind="ExternalInput")
    wT = nc.dram_tensor("wT", (K, O_loc), f32, kind="ExternalInput")
    bias = nc.dram_tensor("bias", (O_loc,), f32, kind="ExternalInput")
    out = nc.dram_tensor("out", (T_loc, O_loc), f32, kind="ExternalOutput")
    stats = nc.dram_tensor("stats", (P, 2), f32, kind="ExternalOutput")

    xT_r = xT.ap().rearrange("(ks p) t -> p ks t", p=P)            # [128,KS,T_loc]
    wT_r = wT.ap().rearrange("(kp s p) o -> kp p s o", s=2, p=P)   # [KP,128,2,O_loc]
    out_r = out.ap().rearrange("(mt p) o -> mt p o", p=P)          # [MT,128,O_loc]

    tape = _emission_tape(MT, NB, KP)

    bf16 = mybir.dt.bfloat16
    with tile.TileContext(nc) as tc:
        with (
            tc.tile_pool(name="const", bufs=1) as const,
            tc.tile_pool(name="xstage", bufs=4) as xstage,
            tc.tile_pool(name="wstage", bufs=3) as wstage,
            tc.tile_pool(name="w8pool", bufs=KP * NB) as w8pool,
            tc.tile_pool(name="x8pool", bufs=MT) as x8pool,
            tc.tile_pool(name="outsb", bufs=3) as outsb,
            tc.tile_pool(name="psum", bufs=8, space="PSUM") as psum,
        ):
            # bias broadcast to all partitions (in place from partition 0)
            bias_sb = const.tile([P, O_loc], f32, name="bias_sb")
            nc.sync.dma_start(bias_sb[0:1, :], bias.ap())
            nc.gpsimd.partition_broadcast(bias_sb[:], bias_sb[0:1, :])

            xpart = const.tile([P, MT * 2], f32, name="xpart")
            wpart = const.tile([P, KP * NB], f32, name="wpart")

            x8 = {}
            w8 = {}
            pending_reduce = []

            def flush_reduce():
                while pending_reduce:
                    r_stg, col = pending_reduce.pop(0)
                    nc.vector.tensor_reduce(
                        xpart[:, col : col + 1], r_stg[:],
                        axis=mybir.AxisListType.XY,
                        op=Alu.max, apply_absolute_value=True,
                    )

            for kind, idx in tape:
                if kind == "x":
                    m, h = idx
                    stg = xstage.tile([P, KH, P], f32, name="xstg")
                    nc.sync.dma_start(
                        stg[:], xT_r[:, h * KH : (h + 1) * KH, m * P : (m + 1) * P]
                    )
                    nc.scalar.activation(stg[:], stg[:], Act.Copy, scale=sx2)
                    if m not in x8:
                        x8[m] = x8pool.tile([P, KS, P], f8, name="x8")
                    nc.vector.tensor_scalar(
                        x8[m][:, h * KH : (h + 1) * KH, :], stg[:],
                        CLIP, -CLIP, op0=Alu.min, op1=Alu.max,
                    )
                    # defer this piece's amax reduce until after the NEXT
                    # piece's clip so the clip (which gates the PE) never
                    # queues behind a 4 us reduce on the DVE stream
                    flush_reduce()
                    pending_reduce.append((stg, 2 * m + h))
                elif kind == "w":
                    j, n = idx
                    stg = wstage.tile([P, 2, 512], f32, name="wstg")
                    nc.sync.dma_start(
                        stg[:], wT_r[j][:, :, n * 512 : (n + 1) * 512]
                    )
                    nc.scalar.activation(stg[:], stg[:], Act.Copy, scale=sw2)
                    w8[(j, n)] = w8pool.tile([P, 2, 512], f8, name="w8")
                    nc.vector.tensor_scalar(
                        w8[(j, n)][:], stg[:], CLIP, -CLIP, op0=Alu.min, op1=Alu.max
                    )
                else:
                    n, m = idx
                    ps = psum.tile([P, 512], f32, name="ps")
                    for j in range(KP):
                        nc.tensor.matmul(
                            ps[:],
                            x8[m][:, 2 * j : 2 * j + 2, :],
                            w8[(j, n)][:],
                            start=(j == 0),
                            stop=(j == KP - 1),
                            perf_mode=mybir.MatmulPerfMode.DoubleRow,
                        )
                    osb = outsb.tile([P, 512], f32, name="osb")
                    nc.vector.scalar_tensor_tensor(
                        osb[:], ps[:], descale,
                        bias_sb[:, n * 512 : (n + 1) * 512],
                        op0=Alu.mult, op1=Alu.add,
                    )
                    nc.sync.dma_start(out_r[m][:, n * 512 : (n + 1) * 512], osb[:])

            flush_reduce()
            # ---- deferred weight amax (on quantized tiles; see kernel())
            # and per-partition stats ----
            for (j, n), t8 in w8.items():
                col = j * NB + n
                nc.vector.tensor_reduce(
                    wpart[:, col : col + 1], t8[:],
                    axis=mybir.AxisListType.XY,
                    op=Alu.max, apply_absolute_value=True,
                )
            st = const.tile([P, 2], f32, name="st")
            nc.vector.tensor_reduce(
                st[:, 0:1], xpart[:], axis=mybir.AxisListType.X, op=Alu.max
            )
            nc.vector.tensor_reduce(
                st[:, 1:2], wpart[:], axis=mybir.AxisListType.X, op=Alu.max
            )
            nc.sync.dma_start(stats.ap(), st[:])
    return nc


def _amax_update(buf, new_amax):
    new_amax = np.float32(new_amax)
    if not np.isfinite(new_amax):
        new_amax = np.float32(AMAX_EPS if not new_amax > 0 else FP8_MAX)
    return np.float32(
        np.clip(np.maximum(np.float32(buf) * np.float32(MOMENTUM), new_amax),
                np.float32(AMAX_EPS), None)
    )


def kernel(x, weight, bias, input_amax, weight_amax):
    x = np.asarray(x, dtype=np.float32)
    weight = np.asarray(weight, dtype=np.float32)
    bias = np.asarray(bias, dtype=np.float32)
    input_amax = np.float32(np.asarray(input_amax))
    weight_amax = np.float32(np.asarray(weight_amax))

    T, K = x.shape
    O = weight.shape[0]
    T_loc, O_loc = T // A_SHARD, O // B_SHARD

    # scales exactly as the reference computes them (f32), then halved (exact)
    sx = np.float32(FP8_MAX) / np.float32(np.clip(input_amax, AMAX_EPS, None))
    sw = np.float32(FP8_MAX) / np.float32(np.clip(weight_amax, AMAX_EPS, None))
    sx2 = np.float32(sx * np.float32(0.5))
    sw2 = np.float32(sw * np.float32(0.5))
    descale = np.float32(1.0 / (np.float64(sx2) * np.float64(sw2)))

    xT = np.ascontiguousarray(x.T)        # [K, T]
    wT = np.ascontiguousarray(weight.T)   # [K, O]

    in_maps = []
    for c in range(N_CORES):
        ai, bi = divmod(c, B_SHARD)
        in_maps.append({
            "xT": xT[:, ai * T_loc : (ai + 1) * T_loc],
            "wT": wT[:, bi * O_loc : (bi + 1) * O_loc],
            "bias": bias[bi * O_loc : (bi + 1) * O_loc],
        })

    nc = bacc.Bacc("TRN2", target_bir_lowering=False, debug=False,
                   num_devices=N_CORES)
    build_kernel(nc, K, T_loc, O_loc, float(sx2), float(sw2), float(descale))
    nc.compile()
    res = bass_utils.run_bass_kernel_spmd(nc, in_maps, core_ids=list(range(N_CORES)))

    out = np.empty((T, O), dtype=np.float32)
    xmax_s = np.float32(0.0)
    wmax_s = np.float32(0.0)
    for c in range(N_CORES):
        ai, bi = divmod(c, B_SHARD)
        r = res.results[c]
        out[ai * T_loc : (ai + 1) * T_loc, bi * O_loc : (bi + 1) * O_loc] = r["out"]
        st = r["stats"]
        xmax_s = max(xmax_s, st[:, 0].max())
        wmax_s = max(wmax_s, st[:, 1].max())

    amax_x = np.float32(np.float64(xmax_s) / np.float64(sx2))
    new_input_amax = _amax_update(input_amax, amax_x)

    # weight amax: the device reduces over the QUANTIZED tiles (saves a DVE
    # pass on the critical path). fp8 RNE is within (1 +- 2^-4) of the true
    # scaled max while unclipped, so when the EMA floor momentum*buf
    # dominates the bound, the reference result is provably momentum*buf and
    # the quantized value is interchangeable. Otherwise fall back to an
    # exact host reduction.
    mom = np.float32(weight_amax) * np.float32(MOMENTUM)
    if wmax_s < 200.0 and np.float64(wmax_s) / np.float64(sw2) * 1.25 < np.float64(mom):
        amax_w = np.float32(np.float64(wmax_s) / np.float64(sw2))
    else:
        amax_w = np.float32(np.abs(weight).max())
    new_weight_amax = _amax_update(weight_amax, amax_w)
    return out, new_input_amax, new_weight_amax


# revision 25
# speedup vs baseline: 1.0268x; 1.0268x over previous
"""FP8 delayed-scaling Linear (8192x4096 @ 4096x4096^T + bias) on 8 NeuronCores.

Strategy: 2D tensor-parallel sharding: token dim (T=8192) split 4 ways x
out_features (O=4096) split 2 ways -> 8 independent cores, no collectives
(the amax max-all-reduce happens in the host-side gather).

Numerics: the reference quantizes with OCP float8_e4m3fn (max 448). TRN2's
fp8e4 has max 240, so we quantize with the HALVED scale (s/2 = 224/amax) and
clip at +-224: every OCP grid point v with |v| <= 448 maps to v/2 which is
exactly representable in TRN fp8e4. The matmul output is then descaled by
4/(sx*sw) with the bias fused into PSUM eviction (scalar_tensor_tensor).

Schedule: one interleaved tape. The first 3 of 4 weight column-groups load
up front (right after the first token strip); each arriving strip then
unlocks three PSUM-bank jobs (~11 us of PE work vs ~10 us of DMA), so the
PE is the binding engine almost immediately. The 4th column group trickles
between strips, and its jobs are spliced in once it completes. Output
stores are spliced into the tape right at job completion so the sync-engine
DMA FIFO never holds finished evictions hostage behind pending input loads.
fp8 DoubleRow matmuls (256-deep contraction per instruction); LDWEIGHTS
hides in the PE reorder window.
"""

import numpy as np

import concourse.bass as bass
import concourse.bacc as bacc
import concourse.mybir as mybir
import concourse.tile as tile
from concourse import bass_utils

P = 128
FP8_MAX = 448.0
AMAX_EPS = 1e-8
MOMENTUM = 0.95
CLIP = 224.0  # 448/2 in scaled units

N_CORES = 8
A_SHARD = 4  # split of T (token rows)
B_SHARD = 2  # split of O (out features)


def _emission_tape(MT, NB, KP):
    """Build the emission tape: ("x", (m, half)), ("w", (j, half)),
    ("job", (n, m)) in input-arrival order with jobs spliced at the point
    their inputs are complete."""
    NH0 = NB - 1 if NB > 1 else NB  # n-blocks in weight column-half 0
    h0 = list(range(NH0))
    h1 = list(range(NH0, NB))

    events = []
    xq = [(m, h) for m in range(MT) for h in range(2)]
    events.extend(("x", xq.pop(0)) for _ in range(min(2, len(xq))))
    for j in range(KP):
        events.append(("w", (j, 0)))
    wrest = [(j, 1) for j in range(KP)] if h1 else []
    while wrest or xq:
        for _ in range(2):
            if xq:
                events.append(("x", xq.pop(0)))
        if wrest:
            events.append(("w", wrest.pop(0)))

    tape = []
    strips_done = []
    wh_count = [0, 0]
    jobs_emitted = 0

    def half_done(h):
        return wh_count[h] == KP

    def emit_jobs_for_strip(m):
        nonlocal jobs_emitted
        if half_done(0):
            for n in h0:
                tape.append(("job", (n, m)))
                jobs_emitted += 1
        if h1 and half_done(1):
            for n in h1:
                tape.append(("job", (n, m)))
                jobs_emitted += 1

    for kind, idx in events:
        tape.append((kind, idx))
        if kind == "x":
            m, h = idx
            if h == 1:
                strips_done.append(m)
                emit_jobs_for_strip(m)
        else:
            j, h = idx
            wh_count[h] += 1
            if half_done(h):
                ns = h0 if h == 0 else h1
                for m in strips_done:
                    for n in ns:
                        tape.append(("job", (n, m)))
                        jobs_emitted += 1
    assert jobs_emitted == NB * MT, (jobs_emitted, NB, MT)
    return tape, NH0


def build_kernel(nc, K, T_loc, O_loc, sx2, sw2, descale):
    """Per-core kernel. Inputs (DRAM): xT [K, T_loc] f32, wT [K, O_loc] f32,
    bias [O_loc] f32. Outputs: out [T_loc, O_loc] f32,
    stats [128, 2] f32 (per-partition max|sx2*x|, max|sw2*w|)."""
    f32 = mybir.dt.float32
    f8 = mybir.dt.float8e4
    Alu = mybir.AluOpType
    Act = mybir.ActivationFunctionType

    assert K % (4 * P) == 0 and T_loc % P == 0 and O_loc % 512 == 0
    KS = K // P          # k-subtiles of 128
    KP = KS // 2         # k-pair slabs (DoubleRow consumes 2 subtiles)
    MT = T_loc // P      # t-strips
    NB = O_loc // 512    # 512-wide n-blocks (one PSUM bank each)
    KH = KS // 2         # k-subtiles per x staging piece

    xT = nc.dram_tensor("xT", (K, T_loc), f32, kind="ExternalInput")
    wT = nc.dram_tensor("wT", (K, O_loc), f32, kind="ExternalInput")
    bias = nc.dram_tensor("bias", (O_loc,), f32, kind="ExternalInput")
    out = nc.dram_tensor("out", (T_loc, O_loc), f32, kind="ExternalOutput")
    stats = nc.dram_tensor("stats", (P, 2), f32, kind="ExternalOutput")

    xT_r = xT.ap().rearrange("(ks p) t -> p ks t", p=P)            # [128,KS,T_loc]
    wT_r = wT.ap().rearrange("(kp s p) o -> kp p s o", s=2, p=P)   # [KP,128,2,O_loc]
    out_r = out.ap().rearrange("(mt p) o -> mt p o", p=P)          # [MT,128,O_loc]

    tape, NH0 = _emission_tape(MT, NB, KP)
    W0 = NH0 * 512               # width of weight column-half 0
    W1 = O_loc - W0

    bf16 = mybir.dt.bfloat16
    with tile.TileContext(nc) as tc:
        with (
            tc.tile_pool(name="const", bufs=1) as const,
            tc.tile_pool(name="xstage", bufs=4) as xstage,
            tc.tile_pool(name="wstage0", bufs=2) as wstage0,
            tc.tile_pool(name="wstage1", bufs=2) as wstage1,
            tc.tile_pool(name="w8p0", bufs=KP) as w8p0,
            tc.tile_pool(name="w8p1", bufs=KP) as w8p1,
            tc.tile_pool(name="x8pool", bufs=MT) as x8pool,
            tc.tile_pool(name="outsb", bufs=3) as outsb,
            tc.tile_pool(name="psum", bufs=8, space="PSUM") as psum,
        ):
            # bias broadcast to all partitions (in place from partition 0)
            bias_sb = const.tile([P, O_loc], f32, name="bias_sb")
            nc.sync.dma_start(bias_sb[0:1, :], bias.ap())
            nc.gpsimd.partition_broadcast(bias_sb[:], bias_sb[0:1, :])

            xpart = const.tile([P, MT * 2], f32, name="xpart")
            wpart = const.tile([P, KP * 2], f32, name="wpart")

            x8 = {}
            w8 = {}
            pending_reduce = []

            def flush_reduce():
                while pending_reduce:
                    r_stg, col = pending_reduce.pop(0)
                    nc.vector.tensor_reduce(
                        xpart[:, col : col + 1], r_stg[:],
                        axis=mybir.AxisListType.XY,
                        op=Alu.max, apply_absolute_value=True,
                    )

            for kind, idx in tape:
                if kind == "x":
                    m, h = idx
                    stg = xstage.tile([P, KH, P], f32, name="xstg")
                    nc.sync.dma_start(
                        stg[:], xT_r[:, h * KH : (h + 1) * KH, m * P : (m + 1) * P]
                    )
                    nc.scalar.activation(stg[:], stg[:], Act.Copy, scale=sx2)
                    if m not in x8:
                        x8[m] = x8pool.tile([P, KS, P], f8, name="x8")
                    nc.vector.tensor_scalar(
                        x8[m][:, h * KH : (h + 1) * KH, :], stg[:],
                        CLIP, -CLIP, op0=Alu.min, op1=Alu.max,
                    )
                    # defer this piece's amax reduce until after the NEXT
                    # piece's clip so the clip (which gates the PE) never
                    # queues behind a 4 us reduce on the DVE stream
                    flush_reduce()
                    pending_reduce.append((stg, 2 * m + h))
                elif kind == "w":
                    j, h = idx
                    lo, wd = (0, W0) if h == 0 else (W0, W1)
                    wstage = wstage0 if h == 0 else wstage1
                    w8pool = w8p0 if h == 0 else w8p1
                    stg = wstage.tile([P, 2, wd], f32, name="wstg")
                    nc.sync.dma_start(stg[:], wT_r[j][:, :, lo : lo + wd])
                    nc.scalar.activation(stg[:], stg[:], Act.Copy, scale=sw2)
                    w8[(j, h)] = w8pool.tile([P, 2, wd], f8, name="w8")
                    nc.vector.tensor_scalar(
                        w8[(j, h)][:], stg[:], CLIP, -CLIP, op0=Alu.min, op1=Alu.max
                    )
                else:
                    n, m = idx
                    h = 0 if n < NH0 else 1
                    off = n * 512 - (0 if h == 0 else W0)
                    ps = psum.tile([P, 512], f32, name="ps")
                    for j in range(KP):
                        nc.tensor.matmul(
                            ps[:],
                            x8[m][:, 2 * j : 2 * j + 2, :],
                            w8[(j, h)][:, :, off : off + 512],
                            start=(j == 0),
                            stop=(j == KP - 1),
                            perf_mode=mybir.MatmulPerfMode.DoubleRow,
                        )
                    osb = outsb.tile([P, 512], f32, name="osb")
                    nc.vector.scalar_tensor_tensor(
                        osb[:], ps[:], descale,
                        bias_sb[:, n * 512 : (n + 1) * 512],
                        op0=Alu.mult, op1=Alu.add,
                    )
                    nc.sync.dma_start(out_r[m][:, n * 512 : (n + 1) * 512], osb[:])

            flush_reduce()
            # ---- deferred weight amax (on quantized tiles; see kernel())
            # and per-partition stats ----
            for (j, h), t8 in w8.items():
                nc.vector.tensor_reduce(
                    wpart[:, 2 * j + h : 2 * j + h + 1], t8[:],
                    axis=mybir.AxisListType.XY,
                    op=Alu.max, apply_absolute_value=True,
                )
            st = const.tile([P, 2], f32, name="st")
            nc.vector.tensor_reduce(
                st[:, 0:1], xpart[:], axis=mybir.AxisListType.X, op=Alu.max
            )
            nc.vector.tensor_reduce(
                st[:, 1:2], wpart[:], axis=mybir.AxisListType.X, op=Alu.max
            )
            nc.sync.dma_start(stats.ap(), st[:])
    return nc


def _amax_update(buf, new_amax):
    new_amax = np.float32(new_amax)
    if not np.isfinite(new_amax):
        new_amax = np.float32(AMAX_EPS if not new_amax > 0 else FP8_MAX)
    return np.float32(
        np.clip(np.maximum(np.float32(buf) * np.float32(MOMENTUM), new_amax),
                np.float32(AMAX_EPS), None)
    )


def kernel(x, weight, bias, input_amax, weight_amax):
    x = np.asarray(x, dtype=np.float32)
    weight = np.asarray(weight, dtype=np.float32)
    bias = np.asarray(bias, dtype=np.float32)
    input_amax = np.float32(np.asarray(input_amax))
    weight_amax = np.float32(np.asarray(weight_amax))

    T, K = x.shape
    O = weight.shape[0]
    T_loc, O_loc = T // A_SHARD, O // B_SHARD

    # scales exactly as the reference computes them (f32), then halved (exact)
    sx = np.float32(FP8_MAX) / np.float32(np.clip(input_amax, AMAX_EPS, None))
    sw = np.float32(FP8_MAX) / np.float32(np.clip(weight_amax, AMAX_EPS, None))
    sx2 = np.float32(sx * np.float32(0.5))
    sw2 = np.float32(sw * np.float32(0.5))
    descale = np.float32(1.0 / (np.float64(sx2) * np.float64(sw2)))

    xT = np.ascontiguousarray(x.T)        # [K, T]
    wT = np.ascontiguousarray(weight.T)   # [K, O]

    in_maps = []
    for c in range(N_CORES):
        ai, bi = divmod(c, B_SHARD)
        in_maps.append({
            "xT": xT[:, ai * T_loc : (ai + 1) * T_loc],
            "wT": wT[:, bi * O_loc : (bi + 1) * O_loc],
            "bias": bias[bi * O_loc : (bi + 1) * O_loc],
        })

    nc = bacc.Bacc("TRN2", target_bir_lowering=False, debug=False,
                   num_devices=N_CORES)
    build_kernel(nc, K, T_loc, O_loc, float(sx2), float(sw2), float(descale))
    nc.compile()
    res = bass_utils.run_bass_kernel_spmd(nc, in_maps, core_ids=list(range(N_CORES)))

    out = np.empty((T, O), dtype=np.float32)
    xmax_s = np.float32(0.0)
    wmax_s = np.float32(0.0)
    for c in range(N_CORES):
        ai, bi = divmod(c, B_SHARD)
        r = res.results[c]
        out[ai * T_loc : (ai + 1) * T_loc, bi * O_loc : (bi + 1) * O_loc] = r["out"]
        st = r["stats"]
        xmax_s = max(xmax_s, st[:, 0].max())
        wmax_s = max(wmax_s, st[:, 1].max())

    amax_x = np.float32(np.float64(xmax_s) / np.float64(sx2))
    new_input_amax = _amax_update(input_amax, amax_x)

    # weight amax: the device reduces over the QUANTIZED tiles (saves a DVE
    # pass on the critical path). fp8 RNE is within (1 +- 2^-4) of the true
    # scaled max while unclipped, so when the EMA floor momentum*buf
    # dominates the bound, the reference result is provably momentum*buf and
    # the quantized value is interchangeable. Otherwise fall back to an
    # exact host reduction.
    mom = np.float32(weight_amax) * np.float32(MOMENTUM)
    if wmax_s < 200.0 and np.float64(wmax_s) / np.float64(sw2) * 1.25 < np.float64(mom):
        amax_w = np.float32(np.float64(wmax_s) / np.float64(sw2))
    else:
        amax_w = np.float32(np.abs(weight).max())
    new_weight_amax = _amax_update(weight_amax, amax_w)
    return out, new_input_amax, new_weight_amax
